# revision 39
# baseline (speedup 1.0000x reference)
"""Block-causal sparse attention (QKNorm + RoPE) for Trainium2, 8 NeuronCores.

Sharding: batch*head parallel. 2 batches x 16 heads = 32 (b,h) pairs; core c
handles batch c//4, heads 4*(c%4) .. 4*(c%4)+4. The out-projection is
computed as per-core partials over the local 256 head channels for all 512
rows of each query-quarter, then one bf16 ReduceScatter(add) per quarter
sums the partials and scatters each core its 128 output rows. The tensor
engine therefore never waits on a collective: attention -> partial matmuls
-> RS -> output DMA pipelines per quarter under later quarters' compute.

Device pipeline per core (single Tile program):
  P1  qkv projection (x @ W_qkv.T) for the local 12 feature blocks (PE);
      PSUM drained as bf16 by ACT (q,k), gpsimd (v), DVE (squares for RMS)
  P2  RMSNorm (over dh=64) + RoPE on q,k in natural [token, feat] layout in
      bf16 on the vector engine, then PE-transpose (bf16, 1 cyc/row) q,k into
      [dh, token] layout
  P3  per (query-range, head-pair, key-block): scoresT = K_j Q^T on PE
      (one matmul per head), a single exp over both heads on ACT (scale=1/8
      folded in; no max-subtraction -- |score/8| <= 8 because q,k are
      RMS-normalized), PV accumulation with a ones-row appended to V so the
      softmax denominator falls out of the same matmul. The denominator is
      broadcast over the head's 64 rows by a PE rank-1 outer product into the
      free partitions of the same PSUM bank (gpsimd and the DMA rings are
      monopolized by in-flight collectives, so neither can carry this), then
      reciprocal (DVE approx) and a fused normalize+move to attnStack (DVE).
  P4  out-projection partials (contraction over the local 256 channels) for
      all 512 rows of the quarter, reduced across the group by the RS.

The block-causal mask (frames of 128 = tile size) is handled by loop bounds;
the single irregular exclusion (last query frame, first key frame) is handled
by zeroing those probs before the PV matmul.

Matmul operands are bf16 (fp32 matmuls run 4x slower per row); accumulation
stays fp32 in PSUM; softmax statistics stay fp32.
"""

import numpy as np

import concourse.bass as bass
from concourse import bacc
import concourse.mybir as mybir
import concourse.tile as tile
from concourse.masks import make_identity

F32 = mybir.dt.float32
BF16 = mybir.dt.bfloat16

B, L, D = 2, 2048, 1024
H, DH = 16, 64
NT = L // 128        # 16 token tiles
HPC = 4              # heads per core
N_CORES = 8
GROUPS = [[0, 1, 2, 3], [4, 5, 6, 7]]
ROPE_THETA = 10000.0
EPS = 1e-6

FQK = 512            # q+k feature columns per core
FV = 256
FTOT = 768


def build_program(apply_gamma=False, qkv_bias=False):
    nc = bacc.Bacc(num_devices=N_CORES)

    xT = nc.declare_dram_parameter("xT", [D, L], BF16, isOutput=False)
    wq = nc.declare_dram_parameter("wq", [D, FTOT], BF16, isOutput=False)
    wo = nc.declare_dram_parameter("wo", [FV, D], BF16, isOutput=False)
    cosb = nc.declare_dram_parameter("cosb", [128, NT, 8, 32], BF16,
                                     isOutput=False)
    sinb = nc.declare_dram_parameter("sinb", [128, NT, 8, 32], BF16,
                                     isOutput=False)
    if apply_gamma:
        gam = nc.declare_dram_parameter("gam", [8, DH], F32, isOutput=False)
    if qkv_bias:
        bqk = nc.declare_dram_parameter("bqk", [FTOT], F32, isOutput=False)
    # rows [128*r + i] = my shard of query rows [512*r + 128*rank + i]
    out = nc.declare_dram_parameter("out", [L // 4, D], BF16, isOutput=True)

    with tile.TileContext(nc) as tc:
        with (
            tc.tile_pool(name="singles", bufs=1) as singles,
            tc.tile_pool(name="persist", bufs=1) as persist,
            tc.tile_pool(name="dram", bufs=1, space="DRAM") as dram,
        ):
            ident = singles.tile([128, 128], BF16)
            make_identity(nc, ident[:])
            epst = singles.tile([128, 1], F32)
            nc.vector.memset(epst[:], EPS)
            ones64 = singles.tile([1, DH], BF16)
            nc.vector.memset(ones64[:], 1.0)

            # V with ones column appended: [tok%128, tile, head, 65]
            vsb = persist.tile([128, NT, HPC, DH + 1], BF16)
            nc.vector.memset(vsb[:, :, :, DH : DH + 1], 1.0)

            # transposed q,k head-pairs: [2*64 feat, L]
            qTs = [persist.tile([128, L], BF16, tag=f"qTs{i}", name=f"qTs{i}")
                   for i in range(2)]
            kTs = [persist.tile([128, L], BF16, tag=f"kTs{i}", name=f"kTs{i}")
                   for i in range(2)]

            # rope tables (pre-replicated over the 8 q/k heads in HBM);
            # DMAs are issued after quarter 0's x/w loads (P2 needs them late)
            cos8 = persist.tile([128, NT, 8, 32], BF16)
            sin8 = persist.tile([128, NT, 8, 32], BF16)

            if apply_gamma:
                gamt = singles.tile([128, 8, DH], F32)
                nc.sync.dma_start(
                    gamt[:],
                    bass.AP(tensor=gam.tensor, offset=gam[:].offset,
                            ap=[[0, 128]] + gam[:].ap))
            if qkv_bias:
                bqkt = singles.tile([128, FTOT], F32)
                nc.sync.dma_start(
                    bqkt[:],
                    bass.AP(tensor=bqk.tensor, offset=bqk[:].offset,
                            ap=[[0, 128]] + bqk[:].ap))

            # qkv weights, split by K-chunk pairs so matmuls can start
            # early; only the first pair is loaded ahead of quarter 0's x
            wqr = wq[:].rearrange("(kc p) f -> p kc f", p=128)
            wqt = [singles.tile([128, 2, FTOT], BF16, name=f"wqt{wi}")
                   for wi in range(4)]
            nc.sync.dma_start(wqt[0][:, 0], wqr[:, 0, :])
            nc.sync.dma_start(wqt[0][:, 1], wqr[:, 1, :])

            attnStack = [persist.tile([128, L], BF16, tag=f"ast{i}", name=f"ast{i}")
                         for i in range(2)]
            wos = persist.tile([128, 2, D], BF16)

            xTr = xT[:].rearrange("(kc p) l -> p kc l", p=128)

            # ---------------- P1: qkv matmuls for one token-quarter ----------
            def emit_xq(qtr, sbp, fine=False):
                tok0 = qtr * 512
                xqA = sbp.tile([128, 4, 512], BF16, tag="xqA", name=f"xqA{qtr}")
                xqB = sbp.tile([128, 4, 512], BF16, tag="xqB", name=f"xqB{qtr}")
                if fine:
                    for kc in range(4):
                        nc.sync.dma_start(
                            xqA[:, kc], xTr[:, kc, tok0 : tok0 + 512])
                    nc.sync.dma_start(xqB[:], xTr[:, 4:8, tok0 : tok0 + 512])
                else:
                    nc.sync.dma_start(xqA[:], xTr[:, 0:4, tok0 : tok0 + 512])
                    nc.sync.dma_start(xqB[:], xTr[:, 4:8, tok0 : tok0 + 512])
                return xqA, xqB

            def emit_p1_quarter(qtr, sbp, psp, xq=None):
                t0 = qtr * 4
                xqA, xqB = xq if xq is not None else emit_xq(qtr, sbp)

                qkraw = sbp.tile([128, 4, 8, DH], BF16, tag="qkraw",
                                 name=f"qkraw{qtr}")
                sqt = sbp.tile([128, 4, 8, DH], BF16, tag="sqt",
                               name=f"sqt{qtr}")

                for t4 in range(4):
                    t = t0 + t4
                    qv_ps = psp.tile([128, FTOT], F32, tag="qv_ps",
                                     name=f"qv{qtr}_{t4}")
                    qk_ps = qv_ps[:, 0:FQK]
                    v_ps = qv_ps[:, FQK:FTOT]
                    for kc in range(8):
                        lhsT = (xqA if kc < 4 else xqB)[
                            :, kc % 4, t4 * 128 : t4 * 128 + 128]
                        wv = wqt[kc // 2][:, kc % 2]
                        nc.tensor.matmul(
                            qk_ps, lhsT, wv[:, 0:FQK],
                            start=(kc == 0), stop=(kc == 7))
                        nc.tensor.matmul(
                            v_ps, lhsT, wv[:, FQK:FTOT],
                            start=(kc == 0), stop=(kc == 7))
                    if qkv_bias:
                        nc.vector.tensor_add(
                            qkraw[:, t4],
                            qk_ps.rearrange("p (g d) -> p g d", d=DH),
                            bqkt[:, 0:FQK].rearrange("p (g d) -> p g d", d=DH))
                        nc.vector.tensor_add(
                            vsb[:, t, :, 0:DH],
                            v_ps.rearrange("p (g d) -> p g d", d=DH),
                            bqkt[:, FQK:FTOT].rearrange("p (g d) -> p g d", d=DH))
                    else:
                        nc.scalar.copy(
                            qkraw[:, t4],
                            qk_ps.rearrange("p (g d) -> p g d", d=DH))
                        nc.scalar.copy(
                            vsb[:, t, :, 0:DH],
                            v_ps.rearrange("p (g d) -> p g d", d=DH))
                    nc.vector.tensor_mul(
                        sqt[:, t4], qkraw[:, t4], qkraw[:, t4])
                return qkraw, sqt

            # ---- P2: rms + rope + transpose for half a quarter (2 tiles) ----
            # (half granularity lets the PE start a quarter's transposes
            # while the vector engine is still roping the other half)
            def emit_p2_half(qtr, hf, qkraw, sqt, sbp, psp):
                t0 = qtr * 4 + 2 * hf

                # RMS statistics (rope is norm-preserving, so stats from raw)
                ssq = sbp.tile([128, 2, 8], F32, tag="ssq",
                               name=f"ssq{qtr}_{hf}")
                nc.vector.reduce_sum(ssq[:], sqt[:, 2 * hf : 2 * hf + 2],
                                     axis=mybir.AxisListType.X)
                nc.scalar.activation(
                    ssq[:], ssq[:], mybir.ActivationFunctionType.Sqrt,
                    bias=epst[:], scale=1.0 / DH)
                nc.vector.reciprocal(ssq[:], ssq[:])
                rmsb = sbp.tile([128, 2, 8], BF16, tag="rmsb",
                                name=f"rmsb{qtr}_{hf}")
                nc.vector.tensor_copy(rmsb[:], ssq[:])

                qk2 = qkraw[:, 2 * hf : 2 * hf + 2]
                if apply_gamma:
                    gview = bass.AP(
                        tensor=gamt.tensor, offset=gamt[:].offset,
                        ap=[gamt[:].ap[0], [0, 2]] + gamt[:].ap[1:])
                    nc.vector.tensor_mul(qk2, qk2, gview)

                # RoPE (all bf16)
                cq = cos8[:, t0 : t0 + 2]
                sq_ = sin8[:, t0 : t0 + 2]
                qkrot = sbp.tile([128, 2, 8, DH], BF16, tag="qkrot",
                                 name=f"qkrot{qtr}_{hf}")
                q1 = qk2[:, :, :, 0:32]
                q2 = qk2[:, :, :, 32:64]
                mA = sbp.tile([128, 2, 8, 32], BF16, tag="mA",
                              name=f"mA{qtr}_{hf}")
                mB = sbp.tile([128, 2, 8, 32], BF16, tag="mB",
                              name=f"mB{qtr}_{hf}")
                nc.vector.tensor_mul(mA[:], q1, cq)
                nc.vector.tensor_mul(mB[:], q2, sq_)
                nc.vector.tensor_sub(qkrot[:, :, :, 0:32], mA[:], mB[:])
                mC = sbp.tile([128, 2, 8, 32], BF16, tag="mA",
                              name=f"mC{qtr}_{hf}")
                mD = sbp.tile([128, 2, 8, 32], BF16, tag="mB",
                              name=f"mD{qtr}_{hf}")
                nc.vector.tensor_mul(mC[:], q2, cq)
                nc.vector.tensor_mul(mD[:], q1, sq_)
                nc.vector.tensor_add(qkrot[:, :, :, 32:64], mC[:], mD[:])

                # apply 1/rms (broadcast [128,2,8] over dh) on gpsimd
                rview = bass.AP(
                    tensor=rmsb.tensor, offset=rmsb[:].offset,
                    ap=rmsb[:].ap + [[0, DH]])
                nc.gpsimd.tensor_mul(qkrot[:], qkrot[:], rview)

                # transpose pairs into qTs/kTs; the two token tiles share one
                # psum tile so the (DVE) drain copies are [128, 256]
                for pr in range(4):
                    dst = (qTs if pr < 2 else kTs)[pr % 2]
                    tp = psp.tile([128, 2, 128], BF16, tag="tp",
                                  name=f"tp{qtr}_{hf}_{pr}")
                    for t2 in range(2):
                        nc.tensor.transpose(
                            tp[:, t2],
                            qkrot[:, t2, 2 * pr : 2 * pr + 2, :],
                            ident[:])
                    nc.vector.tensor_copy(
                        dst[:, t0 * 128 : (t0 + 2) * 128], tp[:])

            def emit_p2_quarter(qtr, qkraw, sqt, sbp, psp):
                emit_p2_half(qtr, 0, qkraw, sqt, sbp, psp)
                emit_p2_half(qtr, 1, qkraw, sqt, sbp, psp)

            # --- P3: one head-pair of one query column-range (<=512 cols) ---
            def emit_p3(r, pair, scps, atps, pbsb, recp, denbp,
                        c0=0, cw=512):
                glo = 512 * r + c0          # global col start
                jmax = (glo + cw) // 128
                kt = kTs[pair]
                qt = qTs[pair]
                ats = [atps.tile([128, 512], F32, tag="at",
                                 name=f"at{r}_{pair}_{c0}_{hi}")
                       for hi in range(2)]
                for j in range(jmax):
                    wlo = max(j * 128 - glo, 0)  # col start within range
                    sc = scps.tile([128, 2, 512], F32, tag="sc",
                                   name=f"sc{r}_{pair}_{c0}_{j}")
                    for hi in range(2):
                        nc.tensor.matmul(
                            sc[:, hi, wlo:cw],
                            kt[64 * hi : 64 * hi + 64,
                               j * 128 : (j + 1) * 128],
                            qt[64 * hi : 64 * hi + 64,
                               glo + wlo : glo + cw],
                            start=True, stop=True)
                    pb = pbsb.tile([128, 2, 512], BF16, tag="pb",
                                   name=f"pb{r}_{pair}_{c0}_{j}")
                    nc.scalar.activation(
                        pb[:, :, wlo:cw], sc[:, :, wlo:cw],
                        mybir.ActivationFunctionType.Exp, scale=1.0 / 8.0)
                    if r == 3 and j == 0:
                        # mask: last query frame can't see key frame 0
                        nc.vector.memset(pb[:, :, 384:512], 0.0)
                    for hi in range(2):
                        nc.tensor.matmul(
                            ats[hi][0 : DH + 1, wlo:cw],
                            vsb[:, j, 2 * pair + hi, :],
                            pb[:, hi, wlo:cw],
                            start=(j == 0), stop=(j == jmax - 1))
                # denominator reciprocal (fast approx, ~18 bits); the PE
                # broadcasts it over 64 rows with a rank-1 outer product into
                # the free partitions of the same at bank (no gpsimd / DMA —
                # both are monopolized by in-flight collectives); ACT moves
                # it to SBUF and DVE does the fused normalize+move.
                for hi in range(2):
                    at = ats[hi]
                    den = recp.tile([1, 512], BF16, tag="den",
                                    name=f"den{r}_{pair}_{c0}_{hi}")
                    nc.scalar.copy(den[0:1, 0:cw], at[DH : DH + 1, 0:cw])
                    nc.tensor.matmul(
                        at[64:128, 0:cw], ones64[:], den[0:1, 0:cw],
                        start=True, stop=True)
                    den64 = denbp.tile([64, 512], F32, tag="den64",
                                       name=f"den64{r}_{pair}_{c0}_{hi}")
                    nc.scalar.copy(den64[:, 0:cw], at[64:128, 0:cw])
                    denb = denbp.tile([64, 512], F32, tag="denb",
                                      name=f"denb{r}_{pair}_{c0}_{hi}")
                    nc.vector.reciprocal_approx_fast(denb[:, 0:cw],
                                                     den64[:, 0:cw])
                    nc.vector.tensor_mul(
                        attnStack[pair][64 * hi : 64 * hi + 64,
                                        glo : glo + cw],
                        at[0:DH, 0:cw], denb[:, 0:cw])

            # ---- P4: out-projection partials + ReduceScatter + out DMA -----
            # partial[rt-th 128 rows of the quarter] over my 256 channels;
            # RS(add) over the group sums partials and hands each core the
            # 128 rows it owns (slot = rank), position-independently.
            def emit_p4(r, osb, scps, split=False):
                ost = osb.tile([128, 4, D], BF16, tag="ost", name=f"ost{r}")
                rs_ins = []
                for o0 in range(0, D, 512):
                    for rt in range(4):
                        op = scps.tile([128, 512], F32, tag="sc",
                                       name=f"op{r}_{rt}_{o0}")
                        for pair in range(2):
                            nc.tensor.matmul(
                                op[:],
                                attnStack[pair][
                                    :, 512 * r + 128 * rt
                                    : 512 * r + 128 * rt + 128],
                                wos[:, pair, o0 : o0 + 512],
                                start=(pair == 0), stop=(pair == 1))
                        nc.vector.tensor_copy(
                            ost[:, rt, o0 : o0 + 512], op[:])
                    if split:
                        rsi = dram.tile([4, 128, 512], BF16,
                                        name=f"rsi{r}_{o0}")
                        nc.sync.dma_start(
                            rsi[:].rearrange("s p o -> p s o"),
                            ost[:, :, o0 : o0 + 512])
                        rs_ins.append(rsi)
                if not split:
                    rsi = dram.tile([4, 128, D], BF16, name=f"rsi{r}")
                    nc.sync.dma_start(
                        rsi[:].rearrange("s p o -> p s o"), ost[:])
                    rs_ins.append(rsi)
                return rs_ins

            # collective triggers are emitted one quarter late so the gpsimd
            # queue (which also carries the partition_broadcasts) never
            # blocks on a previous collective's completion
            def emit_p4_half(r, c0, osb, scps):
                ost = osb.tile([128, 2, D], BF16, tag="ost",
                               name=f"osth{r}_{c0}")
                for o0 in range(0, D, 512):
                    for rt in range(2):
                        op = scps.tile([128, 512], F32, tag="sc",
                                       name=f"oph{r}_{c0}_{rt}_{o0}")
                        for pair in range(2):
                            nc.tensor.matmul(
                                op[:],
                                attnStack[pair][
                                    :, 512 * r + c0 + 128 * rt
                                    : 512 * r + c0 + 128 * rt + 128],
                                wos[:, pair, o0 : o0 + 512],
                                start=(pair == 0), stop=(pair == 1))
                        nc.vector.tensor_copy(
                            ost[:, rt, o0 : o0 + 512], op[:])
                rsi = dram.tile([4, 64, D], BF16, name=f"rsih{r}_{c0}")
                for p in range(4):
                    nc.sync.dma_start(
                        rsi[p], ost[64 * (p % 2) : 64 * (p % 2) + 64,
                                    p // 2, :])
                return rsi

            def emit_rs(r, rs_in, tag=""):
                rows, w = rs_in.shape[1], rs_in.shape[2]
                rs_out = dram.tile([rows, w], BF16, name=f"rso{r}{tag}")
                nc.gpsimd.collective_compute(
                    "ReduceScatter", mybir.AluOpType.add,
                    replica_groups=GROUPS,
                    ins=[rs_in[:].opt()], outs=[rs_out[:].opt()])
                return rs_out

            # ---------------- emission schedule ----------------
            # P1 matmuls run ahead of the (vector-bound) P2 of the previous
            # quarter so the tensor queue never waits on rope.
            with (
                tc.tile_pool(name="p12sb", bufs=2) as p12sb,
                tc.tile_pool(name="p12ps", bufs=2, space="PSUM") as p12ps,
                tc.tile_pool(name="tpps", bufs=2, space="PSUM") as tpps,
            ):
                raws = []
                xq0 = emit_xq(0, p12sb, fine=True)
                for wi in range(1, 4):
                    nc.sync.dma_start(wqt[wi][:],
                                      wqr[:, 2 * wi : 2 * wi + 2, :])
                raws.append(emit_p1_quarter(0, p12sb, p12ps, xq=xq0))
                nc.sync.dma_start(cos8[:], cosb[:])
                nc.sync.dma_start(sin8[:], sinb[:])
                raws.append(emit_p1_quarter(1, p12sb, p12ps))
                emit_p2_quarter(0, *raws[0], p12sb, tpps)
                raws.append(emit_p1_quarter(2, p12sb, p12ps))
                emit_p2_quarter(1, *raws[1], p12sb, tpps)
                raws.append(emit_p1_quarter(3, p12sb, p12ps))
                emit_p2_quarter(2, *raws[2], p12sb, tpps)
                emit_p2_quarter(3, *raws[3], p12sb, tpps)

            nc.scalar.dma_start(
                wos[:], wo[:].rearrange("(pair p) o -> p pair o", p=128))

            with (
                tc.tile_pool(name="scps", bufs=2, space="PSUM") as scps,
                tc.tile_pool(name="atps", bufs=4, space="PSUM") as atps,
                tc.tile_pool(name="pbsb", bufs=4) as pbsb,
                tc.tile_pool(name="recp", bufs=2) as recp,
                tc.tile_pool(name="denbp", bufs=2) as denbp,
                tc.tile_pool(name="osb", bufs=2) as osb,
            ):
                # quarter order [1,2,3,0]: any order is legal once P12 is
                # done; front-loading the heavy quarters starts the collective
                # pipeline earlier so it drains under the remaining compute.
                # quarter 0 (cheapest) is processed last as two row-halves so
                # the final collective is half-size and starts at PE-end.
                # ranges [r3, r0-lo, r1-lo, r2, r1-hi, r0-hi]; each range's
                # out-projection is emitted inside the NEXT range's attention
                # so the PE never idles on the denominator-normalize chain
                rs_outs = {}
                RANGES = [(3, 0, 512), (0, 0, 256), (1, 0, 256),
                          (2, 0, 512), (1, 256, 256), (0, 256, 256)]

                def emit_p4_any(i):
                    r, c0, cw = RANGES[i]
                    if cw == 512:
                        rsi = emit_p4(r, osb, scps)[0]
                        rs_outs[r] = emit_rs(r, rsi)
                    else:
                        rs_outs[(r, c0)] = emit_rs(
                            r, emit_p4_half(r, c0, osb, scps),
                            tag=f"h{r}_{c0}")

                for i, (r, c0, cw) in enumerate(RANGES):
                    emit_p3(r, 0, scps, atps, pbsb, recp, denbp, c0, cw)
                    if 1 <= i <= 3:
                        emit_p4_any(i - 1)
                    emit_p3(r, 1, scps, atps, pbsb, recp, denbp, c0, cw)
                    if i == 4:
                        emit_p4_any(3)
                emit_p4_any(4)
                emit_p4_any(5)
                # output DMAs last: they wait on the collectives, so keeping
                # them out of the mid-stream sync queue avoids a cross-queue
                # convoy (out -> rs_in -> RS trigger -> norm -> PE)
                for r in (2, 3):
                    nc.sync.dma_start(out[128 * r : 128 * (r + 1), :],
                                      rs_outs[r][:])
                for (r, c0) in ((0, 0), (0, 256), (1, 0), (1, 256)):
                    lo = 128 * r + 64 * (c0 // 256)
                    nc.sync.dma_start(out[lo : lo + 64, :],
                                      rs_outs[(r, c0)][:])  # noqa

    nc.compile()
    return nc


_PROG_CACHE = {}


def _get_program(key):
    if key not in _PROG_CACHE:
        _PROG_CACHE[key] = build_program(*key)
    return _PROG_CACHE[key]


def _host_inputs(x, W_qkv, b_qkv, W_out, b_out, q_gamma, k_gamma):
    import ml_dtypes
    mmnp = ml_dtypes.bfloat16
    x = np.asarray(x, dtype=np.float32)
    W_qkv = np.asarray(W_qkv, dtype=np.float32)
    b_qkv = np.asarray(b_qkv, dtype=np.float32)
    W_out = np.asarray(W_out, dtype=np.float32)
    q_gamma = np.asarray(q_gamma, dtype=np.float32)
    k_gamma = np.asarray(k_gamma, dtype=np.float32)

    apply_gamma = not (np.all(q_gamma == 1.0) and np.all(k_gamma == 1.0))
    qkv_bias = bool(np.any(b_qkv))

    # rope tables: pos = t*128 + p, replicated over the 8 q/k head slots
    pos = np.arange(L, dtype=np.float64).reshape(NT, 128).T  # [128, NT]
    inv = 1.0 / (ROPE_THETA ** (np.arange(32, dtype=np.float64) / 32.0))
    ang = pos[:, :, None] * inv[None, None, :]               # [128, NT, 32]
    cosb = np.broadcast_to(
        np.cos(ang)[:, :, None, :], (128, NT, 8, 32)).astype(mmnp).copy()
    sinb = np.broadcast_to(
        np.sin(ang)[:, :, None, :], (128, NT, 8, 32)).astype(mmnp).copy()

    Wq = W_qkv[0 * D : 1 * D]
    Wk = W_qkv[1 * D : 2 * D]
    Wv = W_qkv[2 * D : 3 * D]
    WoT = W_out.T  # [d_in, d_out]

    in_maps = []
    for c in range(N_CORES):
        b = c // 4
        h0 = 4 * (c % 4)
        rows = slice(h0 * DH, (h0 + HPC) * DH)
        wq_c = np.ascontiguousarray(
            np.concatenate([Wq[rows], Wk[rows], Wv[rows]], axis=0).T)
        m = {
            "xT": np.ascontiguousarray(x[b].T).astype(mmnp),
            "wq": wq_c.astype(mmnp),
            "wo": np.ascontiguousarray(WoT[h0 * DH : (h0 + HPC) * DH]
                                       ).astype(mmnp),
            "cosb": cosb,
            "sinb": sinb,
        }
        if apply_gamma:
            m["gam"] = np.ascontiguousarray(
                np.concatenate([np.broadcast_to(q_gamma, (4, DH)),
                                np.broadcast_to(k_gamma, (4, DH))], axis=0))
        if qkv_bias:
            m["bqk"] = np.ascontiguousarray(np.concatenate(
                [b_qkv[0 * D : 1 * D][rows], b_qkv[1 * D : 2 * D][rows],
                 b_qkv[2 * D : 3 * D][rows]]))
        in_maps.append(m)

    key = (apply_gamma, qkv_bias)
    return key, in_maps


def _assemble(results, b_out):
    y = np.empty((B, L, D), dtype=np.float32)
    for c in range(N_CORES):
        b = c // 4
        rank = c % 4
        o = results[c]["out"]
        for r in range(2, 4):
            rows = slice(512 * r + 128 * rank, 512 * r + 128 * rank + 128)
            y[b, rows, :] = o[128 * r : 128 * r + 128].astype(np.float32)
        for r in range(2):
            for h2 in range(2):
                g = 512 * r + 256 * h2 + 64 * rank
                lo = 128 * r + 64 * h2
                y[b, g : g + 64, :] = o[lo : lo + 64].astype(np.float32)
    b_out = np.asarray(b_out, dtype=np.float32)
    if np.any(b_out):
        y += b_out
    return y


def _install_ntff_hook():
    """Register the axon NTFF profiling hook (the container's antenv stub
    lacks axon_hooks; replicate what trn_boot would have registered)."""
    import sys
    import types
    try:
        from antenv.axon_hooks import get_axon_ntff_profile_hook  # noqa: F401
        return
    except ImportError:
        pass
    try:
        from trn_agent_boot.trn_boot import _ntff_profile_via_ctypes
        hook = _ntff_profile_via_ctypes("/opt/axon/libaxon_pjrt.so")
    except Exception:
        hook = None
    import antenv
    mod = types.ModuleType("antenv.axon_hooks")
    mod.get_axon_ntff_profile_hook = lambda: hook
    mod.set_axon_ntff_profile_hook = lambda h: None
    antenv.axon_hooks = mod
    sys.modules["antenv.axon_hooks"] = mod


def kernel(x, W_qkv, b_qkv, W_out, b_out, q_gamma, k_gamma, _trace=False):
    from concourse.bass_utils import run_bass_kernel_spmd
    if _trace:
        _install_ntff_hook()

    key, in_maps = _host_inputs(x, W_qkv, b_qkv, W_out, b_out,
                                q_gamma, k_gamma)
    nc = _get_program(key)
    res = run_bass_kernel_spmd(nc, in_maps, core_ids=list(range(N_CORES)),
                               trace=_trace,
                               trace_cores=list(range(N_CORES)) if _trace else None)
    y = _assemble(res.results, b_out)
    if _trace:
        return y, res
    return y


# revision 40
# speedup vs baseline: 1.1201x; 1.1201x over previous
"""Block-causal sparse attention (QKNorm + RoPE) for Trainium2, 8 NeuronCores.

Sharding: batch*head parallel. 2 batches x 16 heads = 32 (b,h) pairs; core c
handles batch c//4, heads 4*(c%4) .. 4*(c%4)+4. The out-projection is
computed as per-core partials over the local 256 head channels for all 512
rows of each query-quarter, then one bf16 ReduceScatter(add) per quarter
sums the partials and scatters each core its 128 output rows. The tensor
engine therefore never waits on a collective: attention -> partial matmuls
-> RS -> output DMA pipelines per quarter under later quarters' compute.

Device pipeline per core (single Tile program):
  P1  qkv projection (x @ W_qkv.T) for the local 12 feature blocks (PE);
      PSUM drained as bf16 by ACT (q,k), gpsimd (v), DVE (squares for RMS)
  P2  RMSNorm (over dh=64) + RoPE on q,k in natural [token, feat] layout in
      bf16 on the vector engine, then PE-transpose (bf16, 1 cyc/row) q,k into
      [dh, token] layout
  P3  per (query-range, head-pair, key-block): scoresT = K_j Q^T on PE
      (one matmul per head), a single exp over both heads on ACT (scale=1/8
      folded in; no max-subtraction -- |score/8| <= 8 because q,k are
      RMS-normalized), PV accumulation with a ones-row appended to V so the
      softmax denominator falls out of the same matmul. The denominator is
      broadcast over the head's 64 rows by a PE rank-1 outer product into the
      free partitions of the same PSUM bank (gpsimd and the DMA rings are
      monopolized by in-flight collectives, so neither can carry this), then
      reciprocal (DVE approx) and a fused normalize+move to attnStack (DVE).
  P4  out-projection partials (contraction over the local 256 channels) for
      all 512 rows of the quarter, reduced across the group by the RS.

The block-causal mask (frames of 128 = tile size) is handled by loop bounds;
the single irregular exclusion (last query frame, first key frame) is handled
by zeroing those probs before the PV matmul.

Matmul operands are bf16 (fp32 matmuls run 4x slower per row); accumulation
stays fp32 in PSUM; softmax statistics stay fp32.
"""

import numpy as np

import concourse.bass as bass
from concourse import bacc
import concourse.mybir as mybir
import concourse.tile as tile
from concourse.masks import make_identity

F32 = mybir.dt.float32
BF16 = mybir.dt.bfloat16

B, L, D = 2, 2048, 1024
H, DH = 16, 64
NT = L // 128        # 16 token tiles
HPC = 4              # heads per core
N_CORES = 8
GROUPS = [[0, 1, 2, 3], [4, 5, 6, 7]]
ROPE_THETA = 10000.0
EPS = 1e-6

FQK = 512            # q+k feature columns per core
FV = 256
FTOT = 768


def build_program(apply_gamma=False, qkv_bias=False):
    nc = bacc.Bacc(num_devices=N_CORES)

    xT = nc.declare_dram_parameter("xT", [D, L], BF16, isOutput=False)
    wq = nc.declare_dram_parameter("wq", [D, FTOT], BF16, isOutput=False)
    wo = nc.declare_dram_parameter("wo", [FV, D], BF16, isOutput=False)
    cosb = nc.declare_dram_parameter("cosb", [128, NT, 8, 32], BF16,
                                     isOutput=False)
    sinb = nc.declare_dram_parameter("sinb", [128, NT, 8, 32], BF16,
                                     isOutput=False)
    if apply_gamma:
        gam = nc.declare_dram_parameter("gam", [8, DH], F32, isOutput=False)
    if qkv_bias:
        bqk = nc.declare_dram_parameter("bqk", [FTOT], F32, isOutput=False)
    # rows [128*r + i] = my shard of query rows [512*r + 128*rank + i]
    out = nc.declare_dram_parameter("out", [L // 4, D], BF16, isOutput=True)

    with tile.TileContext(nc) as tc:
        with (
            tc.tile_pool(name="singles", bufs=1) as singles,
            tc.tile_pool(name="persist", bufs=1) as persist,
            tc.tile_pool(name="dram", bufs=1, space="DRAM") as dram,
        ):
            ident = singles.tile([128, 128], BF16)
            make_identity(nc, ident[:])
            epst = singles.tile([128, 1], F32)
            nc.vector.memset(epst[:], EPS)
            ones64 = singles.tile([1, DH], BF16)
            nc.vector.memset(ones64[:], 1.0)

            # V with ones column appended: [tok%128, tile, head, 65]
            vsb = persist.tile([128, NT, HPC, DH + 1], BF16)
            nc.vector.memset(vsb[:, :, :, DH : DH + 1], 1.0)

            # transposed q,k head-pairs: [2*64 feat, L]
            qTs = [persist.tile([128, L], BF16, tag=f"qTs{i}", name=f"qTs{i}")
                   for i in range(2)]
            kTs = [persist.tile([128, L], BF16, tag=f"kTs{i}", name=f"kTs{i}")
                   for i in range(2)]

            # rope tables (pre-replicated over the 8 q/k heads in HBM);
            # DMAs are issued after quarter 0's x/w loads (P2 needs them late)
            cos8 = persist.tile([128, NT, 8, 32], BF16)
            sin8 = persist.tile([128, NT, 8, 32], BF16)

            if apply_gamma:
                gamt = singles.tile([128, 8, DH], F32)
                nc.sync.dma_start(
                    gamt[:],
                    bass.AP(tensor=gam.tensor, offset=gam[:].offset,
                            ap=[[0, 128]] + gam[:].ap))
            if qkv_bias:
                bqkt = singles.tile([128, FTOT], F32)
                nc.sync.dma_start(
                    bqkt[:],
                    bass.AP(tensor=bqk.tensor, offset=bqk[:].offset,
                            ap=[[0, 128]] + bqk[:].ap))

            # qkv weights, split by K-chunk pairs so matmuls can start
            # early; only the first pair is loaded ahead of quarter 0's x
            wqr = wq[:].rearrange("(kc p) f -> p kc f", p=128)
            wqt = [singles.tile([128, 2, FTOT], BF16, name=f"wqt{wi}")
                   for wi in range(4)]
            nc.sync.dma_start(wqt[0][:, 0], wqr[:, 0, :])
            nc.sync.dma_start(wqt[0][:, 1], wqr[:, 1, :])

            attnStack = [persist.tile([128, L], BF16, tag=f"ast{i}", name=f"ast{i}")
                         for i in range(2)]
            wos = persist.tile([128, 2, D], BF16)

            xTr = xT[:].rearrange("(kc p) l -> p kc l", p=128)

            # ---------------- P1: qkv matmuls for one token-quarter ----------
            def emit_xq(qtr, sbp, fine=False):
                tok0 = qtr * 512
                xqA = sbp.tile([128, 4, 512], BF16, tag="xqA", name=f"xqA{qtr}")
                xqB = sbp.tile([128, 4, 512], BF16, tag="xqB", name=f"xqB{qtr}")
                if fine:
                    for kc in range(4):
                        nc.sync.dma_start(
                            xqA[:, kc], xTr[:, kc, tok0 : tok0 + 512])
                    nc.sync.dma_start(xqB[:], xTr[:, 4:8, tok0 : tok0 + 512])
                else:
                    nc.sync.dma_start(xqA[:], xTr[:, 0:4, tok0 : tok0 + 512])
                    nc.sync.dma_start(xqB[:], xTr[:, 4:8, tok0 : tok0 + 512])
                return xqA, xqB

            def emit_p1_quarter(qtr, sbp, psp, xq=None):
                t0 = qtr * 4
                xqA, xqB = xq if xq is not None else emit_xq(qtr, sbp)

                qkraw = sbp.tile([128, 4, 8, DH], BF16, tag="qkraw",
                                 name=f"qkraw{qtr}")
                sqt = sbp.tile([128, 4, 8, DH], BF16, tag="sqt",
                               name=f"sqt{qtr}")

                for t4 in range(4):
                    t = t0 + t4
                    qv_ps = psp.tile([128, FTOT], F32, tag="qv_ps",
                                     name=f"qv{qtr}_{t4}")
                    qk_ps = qv_ps[:, 0:FQK]
                    v_ps = qv_ps[:, FQK:FTOT]
                    for kc in range(8):
                        lhsT = (xqA if kc < 4 else xqB)[
                            :, kc % 4, t4 * 128 : t4 * 128 + 128]
                        wv = wqt[kc // 2][:, kc % 2]
                        nc.tensor.matmul(
                            qk_ps, lhsT, wv[:, 0:FQK],
                            start=(kc == 0), stop=(kc == 7))
                        nc.tensor.matmul(
                            v_ps, lhsT, wv[:, FQK:FTOT],
                            start=(kc == 0), stop=(kc == 7))
                    if qkv_bias:
                        nc.vector.tensor_add(
                            qkraw[:, t4],
                            qk_ps.rearrange("p (g d) -> p g d", d=DH),
                            bqkt[:, 0:FQK].rearrange("p (g d) -> p g d", d=DH))
                        nc.vector.tensor_add(
                            vsb[:, t, :, 0:DH],
                            v_ps.rearrange("p (g d) -> p g d", d=DH),
                            bqkt[:, FQK:FTOT].rearrange("p (g d) -> p g d", d=DH))
                    else:
                        nc.scalar.copy(
                            qkraw[:, t4],
                            qk_ps.rearrange("p (g d) -> p g d", d=DH))
                        nc.scalar.copy(
                            vsb[:, t, :, 0:DH],
                            v_ps.rearrange("p (g d) -> p g d", d=DH))
                    nc.vector.tensor_mul(
                        sqt[:, t4], qkraw[:, t4], qkraw[:, t4])
                return qkraw, sqt

            # ---- P2: rms + rope + transpose for half a quarter (2 tiles) ----
            # (half granularity lets the PE start a quarter's transposes
            # while the vector engine is still roping the other half)
            def emit_p2_half(qtr, hf, qkraw, sqt, sbp, psp):
                t0 = qtr * 4 + 2 * hf

                # RMS statistics (rope is norm-preserving, so stats from raw)
                ssq = sbp.tile([128, 2, 8], F32, tag="ssq",
                               name=f"ssq{qtr}_{hf}")
                nc.vector.reduce_sum(ssq[:], sqt[:, 2 * hf : 2 * hf + 2],
                                     axis=mybir.AxisListType.X)
                nc.scalar.activation(
                    ssq[:], ssq[:], mybir.ActivationFunctionType.Sqrt,
                    bias=epst[:], scale=1.0 / DH)
                nc.vector.reciprocal(ssq[:], ssq[:])
                rmsb = sbp.tile([128, 2, 8], BF16, tag="rmsb",
                                name=f"rmsb{qtr}_{hf}")
                nc.vector.tensor_copy(rmsb[:], ssq[:])

                qk2 = qkraw[:, 2 * hf : 2 * hf + 2]
                if apply_gamma:
                    gview = bass.AP(
                        tensor=gamt.tensor, offset=gamt[:].offset,
                        ap=[gamt[:].ap[0], [0, 2]] + gamt[:].ap[1:])
                    nc.vector.tensor_mul(qk2, qk2, gview)

                # RoPE (all bf16)
                cq = cos8[:, t0 : t0 + 2]
                sq_ = sin8[:, t0 : t0 + 2]
                qkrot = sbp.tile([128, 2, 8, DH], BF16, tag="qkrot",
                                 name=f"qkrot{qtr}_{hf}")
                q1 = qk2[:, :, :, 0:32]
                q2 = qk2[:, :, :, 32:64]
                mA = sbp.tile([128, 2, 8, 32], BF16, tag="mA",
                              name=f"mA{qtr}_{hf}")
                mB = sbp.tile([128, 2, 8, 32], BF16, tag="mB",
                              name=f"mB{qtr}_{hf}")
                nc.vector.tensor_mul(mA[:], q1, cq)
                nc.vector.tensor_mul(mB[:], q2, sq_)
                nc.vector.tensor_sub(qkrot[:, :, :, 0:32], mA[:], mB[:])
                mC = sbp.tile([128, 2, 8, 32], BF16, tag="mA",
                              name=f"mC{qtr}_{hf}")
                mD = sbp.tile([128, 2, 8, 32], BF16, tag="mB",
                              name=f"mD{qtr}_{hf}")
                nc.vector.tensor_mul(mC[:], q2, cq)
                nc.vector.tensor_mul(mD[:], q1, sq_)
                nc.vector.tensor_add(qkrot[:, :, :, 32:64], mC[:], mD[:])

                # apply 1/rms (broadcast [128,2,8] over dh) on gpsimd
                rview = bass.AP(
                    tensor=rmsb.tensor, offset=rmsb[:].offset,
                    ap=rmsb[:].ap + [[0, DH]])
                nc.gpsimd.tensor_mul(qkrot[:], qkrot[:], rview)

                # transpose pairs into qTs/kTs; the two token tiles share one
                # psum tile so the (DVE) drain copies are [128, 256]
                for pr in range(4):
                    dst = (qTs if pr < 2 else kTs)[pr % 2]
                    tp = psp.tile([128, 2, 128], BF16, tag="tp",
                                  name=f"tp{qtr}_{hf}_{pr}")
                    for t2 in range(2):
                        nc.tensor.transpose(
                            tp[:, t2],
                            qkrot[:, t2, 2 * pr : 2 * pr + 2, :],
                            ident[:])
                    nc.vector.tensor_copy(
                        dst[:, t0 * 128 : (t0 + 2) * 128], tp[:])

            def emit_p2_quarter(qtr, qkraw, sqt, sbp, psp):
                emit_p2_half(qtr, 0, qkraw, sqt, sbp, psp)
                emit_p2_half(qtr, 1, qkraw, sqt, sbp, psp)

            # --- P3: one head-pair of one query column-range (<=512 cols) ---
            def emit_p3(r, pair, scps, atps, pbsb, recp, denbp,
                        c0=0, cw=512):
                glo = 512 * r + c0          # global col start
                jmax = (glo + cw) // 128
                kt = kTs[pair]
                qt = qTs[pair]
                ats = [atps.tile([128, 512], F32, tag="at",
                                 name=f"at{r}_{pair}_{c0}_{hi}")
                       for hi in range(2)]
                for j in range(jmax):
                    wlo = max(j * 128 - glo, 0)  # col start within range
                    sc = scps.tile([128, 2, 512], F32, tag="sc",
                                   name=f"sc{r}_{pair}_{c0}_{j}")
                    for hi in range(2):
                        nc.tensor.matmul(
                            sc[:, hi, wlo:cw],
                            kt[64 * hi : 64 * hi + 64,
                               j * 128 : (j + 1) * 128],
                            qt[64 * hi : 64 * hi + 64,
                               glo + wlo : glo + cw],
                            start=True, stop=True)
                    pb = pbsb.tile([128, 2, 512], BF16, tag="pb",
                                   name=f"pb{r}_{pair}_{c0}_{j}")
                    nc.scalar.activation(
                        pb[:, :, wlo:cw], sc[:, :, wlo:cw],
                        mybir.ActivationFunctionType.Exp, scale=1.0 / 8.0)
                    if r == 3 and j == 0:
                        # mask: last query frame can't see key frame 0
                        nc.vector.memset(pb[:, :, 384:512], 0.0)
                    for hi in range(2):
                        nc.tensor.matmul(
                            ats[hi][0 : DH + 1, wlo:cw],
                            vsb[:, j, 2 * pair + hi, :],
                            pb[:, hi, wlo:cw],
                            start=(j == 0), stop=(j == jmax - 1))
                # denominator reciprocal (fast approx, ~18 bits); the PE
                # broadcasts it over 64 rows with a rank-1 outer product into
                # the free partitions of the same at bank (no gpsimd / DMA —
                # both are monopolized by in-flight collectives); ACT moves
                # it to SBUF and DVE does the fused normalize+move.
                for hi in range(2):
                    at = ats[hi]
                    den = recp.tile([1, 512], BF16, tag="den",
                                    name=f"den{r}_{pair}_{c0}_{hi}")
                    nc.scalar.copy(den[0:1, 0:cw], at[DH : DH + 1, 0:cw])
                    nc.tensor.matmul(
                        at[64:128, 0:cw], ones64[:], den[0:1, 0:cw],
                        start=True, stop=True)
                    den64 = denbp.tile([64, 512], F32, tag="den64",
                                       name=f"den64{r}_{pair}_{c0}_{hi}")
                    nc.scalar.copy(den64[:, 0:cw], at[64:128, 0:cw])
                    denb = denbp.tile([64, 512], F32, tag="denb",
                                      name=f"denb{r}_{pair}_{c0}_{hi}")
                    nc.vector.reciprocal_approx_fast(denb[:, 0:cw],
                                                     den64[:, 0:cw])
                    nc.vector.tensor_mul(
                        attnStack[pair][64 * hi : 64 * hi + 64,
                                        glo : glo + cw],
                        at[0:DH, 0:cw], denb[:, 0:cw])

            # ---- P4: out-projection partials + ReduceScatter + out DMA -----
            # partial[rt-th 128 rows of the quarter] over my 256 channels;
            # RS(add) over the group sums partials and hands each core the
            # 128 rows it owns (slot = rank), position-independently.
            def emit_p4(r, osb, scps, split=False):
                ost = osb.tile([128, 4, D], BF16, tag="ost", name=f"ost{r}")
                rs_ins = []
                for o0 in range(0, D, 512):
                    for rt in range(4):
                        op = scps.tile([128, 512], F32, tag="sc",
                                       name=f"op{r}_{rt}_{o0}")
                        for pair in range(2):
                            nc.tensor.matmul(
                                op[:],
                                attnStack[pair][
                                    :, 512 * r + 128 * rt
                                    : 512 * r + 128 * rt + 128],
                                wos[:, pair, o0 : o0 + 512],
                                start=(pair == 0), stop=(pair == 1))
                        nc.vector.tensor_copy(
                            ost[:, rt, o0 : o0 + 512], op[:])
                    if split:
                        rsi = dram.tile([4, 128, 512], BF16,
                                        name=f"rsi{r}_{o0}")
                        nc.sync.dma_start(
                            rsi[:].rearrange("s p o -> p s o"),
                            ost[:, :, o0 : o0 + 512])
                        rs_ins.append(rsi)
                if not split:
                    rsi = dram.tile([4, 128, D], BF16, name=f"rsi{r}")
                    nc.sync.dma_start(
                        rsi[:].rearrange("s p o -> p s o"), ost[:])
                    rs_ins.append(rsi)
                return rs_ins

            # collective triggers are emitted one quarter late so the gpsimd
            # queue (which also carries the partition_broadcasts) never
            # blocks on a previous collective's completion
            def emit_p4_half(r, c0, osb, scps):
                ost = osb.tile([128, 2, D], BF16, tag="ost",
                               name=f"osth{r}_{c0}")
                for o0 in range(0, D, 512):
                    for rt in range(2):
                        op = scps.tile([128, 512], F32, tag="sc",
                                       name=f"oph{r}_{c0}_{rt}_{o0}")
                        for pair in range(2):
                            nc.tensor.matmul(
                                op[:],
                                attnStack[pair][
                                    :, 512 * r + c0 + 128 * rt
                                    : 512 * r + c0 + 128 * rt + 128],
                                wos[:, pair, o0 : o0 + 512],
                                start=(pair == 0), stop=(pair == 1))
                        nc.vector.tensor_copy(
                            ost[:, rt, o0 : o0 + 512], op[:])
                rsi = dram.tile([4, 64, D], BF16, name=f"rsih{r}_{c0}")
                for p in range(4):
                    nc.sync.dma_start(
                        rsi[p], ost[64 * (p % 2) : 64 * (p % 2) + 64,
                                    p // 2, :])
                return rsi

            def emit_rs(r, rs_in, tag=""):
                rows, w = rs_in.shape[1], rs_in.shape[2]
                rs_out = dram.tile([rows, w], BF16, name=f"rso{r}{tag}")
                nc.gpsimd.collective_compute(
                    "ReduceScatter", mybir.AluOpType.add,
                    replica_groups=GROUPS,
                    ins=[rs_in[:].opt()], outs=[rs_out[:].opt()])
                return rs_out

            # ---------------- emission schedule ----------------
            # P1 matmuls run ahead of the (vector-bound) P2 of the previous
            # quarter so the tensor queue never waits on rope.
            with (
                tc.tile_pool(name="p12sb", bufs=2) as p12sb,
                tc.tile_pool(name="p12ps", bufs=2, space="PSUM") as p12ps,
                tc.tile_pool(name="tpps", bufs=2, space="PSUM") as tpps,
            ):
                raws = []
                xq0 = emit_xq(0, p12sb, fine=True)
                for wi in range(1, 4):
                    nc.sync.dma_start(wqt[wi][:],
                                      wqr[:, 2 * wi : 2 * wi + 2, :])
                raws.append(emit_p1_quarter(0, p12sb, p12ps, xq=xq0))
                nc.sync.dma_start(cos8[:], cosb[:])
                nc.sync.dma_start(sin8[:], sinb[:])
                raws.append(emit_p1_quarter(1, p12sb, p12ps))
                emit_p2_quarter(0, *raws[0], p12sb, tpps)
                raws.append(emit_p1_quarter(2, p12sb, p12ps))
                emit_p2_quarter(1, *raws[1], p12sb, tpps)
                raws.append(emit_p1_quarter(3, p12sb, p12ps))
                emit_p2_quarter(2, *raws[2], p12sb, tpps)
                emit_p2_quarter(3, *raws[3], p12sb, tpps)

            nc.scalar.dma_start(
                wos[:], wo[:].rearrange("(pair p) o -> p pair o", p=128))

            with (
                tc.tile_pool(name="scps", bufs=3, space="PSUM") as scps,
                tc.tile_pool(name="atps", bufs=2, space="PSUM") as atps,
                tc.tile_pool(name="pbsb", bufs=4) as pbsb,
                tc.tile_pool(name="recp", bufs=2) as recp,
                tc.tile_pool(name="denbp", bufs=2) as denbp,
                tc.tile_pool(name="osb", bufs=2) as osb,
            ):
                # quarter order [1,2,3,0]: any order is legal once P12 is
                # done; front-loading the heavy quarters starts the collective
                # pipeline earlier so it drains under the remaining compute.
                # quarter 0 (cheapest) is processed last as two row-halves so
                # the final collective is half-size and starts at PE-end.
                rs_outs = {}
                for (r, c0) in ((0, 0), (1, 0)):
                    emit_p3(r, 0, scps, atps, pbsb, recp, denbp, c0, 256)
                    emit_p3(r, 1, scps, atps, pbsb, recp, denbp, c0, 256)
                    rs_outs[(r, c0)] = emit_rs(
                        r, emit_p4_half(r, c0, osb, scps), tag=f"h{r}_{c0}")
                for r in (3, 2):
                    emit_p3(r, 0, scps, atps, pbsb, recp, denbp)
                    emit_p3(r, 1, scps, atps, pbsb, recp, denbp)
                    rsi = emit_p4(r, osb, scps)[0]
                    rs_outs[r] = emit_rs(r, rsi)
                for (r, c0) in ((1, 256), (0, 256)):
                    emit_p3(r, 0, scps, atps, pbsb, recp, denbp, c0, 256)
                    emit_p3(r, 1, scps, atps, pbsb, recp, denbp, c0, 256)
                    rs_outs[(r, c0)] = emit_rs(
                        r, emit_p4_half(r, c0, osb, scps), tag=f"h{r}_{c0}")
                # output DMAs last: they wait on the collectives, so keeping
                # them out of the mid-stream sync queue avoids a cross-queue
                # convoy (out -> rs_in -> RS trigger -> norm -> PE)
                for r in (2, 3):
                    nc.sync.dma_start(out[128 * r : 128 * (r + 1), :],
                                      rs_outs[r][:])
                for (r, c0) in ((0, 0), (0, 256), (1, 0), (1, 256)):
                    lo = 128 * r + 64 * (c0 // 256)
                    nc.sync.dma_start(out[lo : lo + 64, :],
                                      rs_outs[(r, c0)][:])  # noqa

    nc.compile()
    return nc


_PROG_CACHE = {}


def _get_program(key):
    if key not in _PROG_CACHE:
        _PROG_CACHE[key] = build_program(*key)
    return _PROG_CACHE[key]


def _host_inputs(x, W_qkv, b_qkv, W_out, b_out, q_gamma, k_gamma):
    import ml_dtypes
    mmnp = ml_dtypes.bfloat16
    x = np.asarray(x, dtype=np.float32)
    W_qkv = np.asarray(W_qkv, dtype=np.float32)
    b_qkv = np.asarray(b_qkv, dtype=np.float32)
    W_out = np.asarray(W_out, dtype=np.float32)
    q_gamma = np.asarray(q_gamma, dtype=np.float32)
    k_gamma = np.asarray(k_gamma, dtype=np.float32)

    apply_gamma = not (np.all(q_gamma == 1.0) and np.all(k_gamma == 1.0))
    qkv_bias = bool(np.any(b_qkv))

    # rope tables: pos = t*128 + p, replicated over the 8 q/k head slots
    pos = np.arange(L, dtype=np.float64).reshape(NT, 128).T  # [128, NT]
    inv = 1.0 / (ROPE_THETA ** (np.arange(32, dtype=np.float64) / 32.0))
    ang = pos[:, :, None] * inv[None, None, :]               # [128, NT, 32]
    cosb = np.broadcast_to(
        np.cos(ang)[:, :, None, :], (128, NT, 8, 32)).astype(mmnp).copy()
    sinb = np.broadcast_to(
        np.sin(ang)[:, :, None, :], (128, NT, 8, 32)).astype(mmnp).copy()

    Wq = W_qkv[0 * D : 1 * D]
    Wk = W_qkv[1 * D : 2 * D]
    Wv = W_qkv[2 * D : 3 * D]
    WoT = W_out.T  # [d_in, d_out]

    in_maps = []
    for c in range(N_CORES):
        b = c // 4
        h0 = 4 * (c % 4)
        rows = slice(h0 * DH, (h0 + HPC) * DH)
        wq_c = np.ascontiguousarray(
            np.concatenate([Wq[rows], Wk[rows], Wv[rows]], axis=0).T)
        m = {
            "xT": np.ascontiguousarray(x[b].T).astype(mmnp),
            "wq": wq_c.astype(mmnp),
            "wo": np.ascontiguousarray(WoT[h0 * DH : (h0 + HPC) * DH]
                                       ).astype(mmnp),
            "cosb": cosb,
            "sinb": sinb,
        }
        if apply_gamma:
            m["gam"] = np.ascontiguousarray(
                np.concatenate([np.broadcast_to(q_gamma, (4, DH)),
                                np.broadcast_to(k_gamma, (4, DH))], axis=0))
        if qkv_bias:
            m["bqk"] = np.ascontiguousarray(np.concatenate(
                [b_qkv[0 * D : 1 * D][rows], b_qkv[1 * D : 2 * D][rows],
                 b_qkv[2 * D : 3 * D][rows]]))
        in_maps.append(m)

    key = (apply_gamma, qkv_bias)
    return key, in_maps


def _assemble(results, b_out):
    y = np.empty((B, L, D), dtype=np.float32)
    for c in range(N_CORES):
        b = c // 4
        rank = c % 4
        o = results[c]["out"]
        for r in range(2, 4):
            rows = slice(512 * r + 128 * rank, 512 * r + 128 * rank + 128)
            y[b, rows, :] = o[128 * r : 128 * r + 128].astype(np.float32)
        for r in range(2):
            for h2 in range(2):
                g = 512 * r + 256 * h2 + 64 * rank
                lo = 128 * r + 64 * h2
                y[b, g : g + 64, :] = o[lo : lo + 64].astype(np.float32)
    b_out = np.asarray(b_out, dtype=np.float32)
    if np.any(b_out):
        y += b_out
    return y


def _install_ntff_hook():
    """Register the axon NTFF profiling hook (the container's antenv stub
    lacks axon_hooks; replicate what trn_boot would have registered)."""
    import sys
    import types
    try:
        from antenv.axon_hooks import get_axon_ntff_profile_hook  # noqa: F401
        return
    except ImportError:
        pass
    try:
        from trn_agent_boot.trn_boot import _ntff_profile_via_ctypes
        hook = _ntff_profile_via_ctypes("/opt/axon/libaxon_pjrt.so")
    except Exception:
        hook = None
    import antenv
    mod = types.ModuleType("antenv.axon_hooks")
    mod.get_axon_ntff_profile_hook = lambda: hook
    mod.set_axon_ntff_profile_hook = lambda h: None
    antenv.axon_hooks = mod
    sys.modules["antenv.axon_hooks"] = mod


def kernel(x, W_qkv, b_qkv, W_out, b_out, q_gamma, k_gamma, _trace=False):
    from concourse.bass_utils import run_bass_kernel_spmd
    if _trace:
        _install_ntff_hook()

    key, in_maps = _host_inputs(x, W_qkv, b_qkv, W_out, b_out,
                                q_gamma, k_gamma)
    nc = _get_program(key)
    res = run_bass_kernel_spmd(nc, in_maps, core_ids=list(range(N_CORES)),
                               trace=_trace,
                               trace_cores=list(range(N_CORES)) if _trace else None)
    y = _assemble(res.results, b_out)
    if _trace:
        return y, res
    return y
